# revision 4
# baseline (speedup 1.0000x reference)
"""Trainium2 Bass kernel for a dense transformer block (RMSNorm + MHA + SwiGLU MLP).

Sharding: sequence-parallel over the 8 cores (batch 0 -> cores 0-3,
batch 1 -> cores 4-7; each core owns 512 tokens).  Each core computes
q/k/v for its own tokens, the K/V shards are exchanged with one
AllGather per tensor inside each 4-core group, and everything else
(attention for the local queries, o-proj, MLP) is computed locally with
full (replicated) weights streamed from HBM.  This gives each core
exactly 1/8 of the total FLOPs with only ~4 MB of collective traffic.

Layouts on device are feature-major ([feature, token]) so that every
matmul is `out[f_out, t] += W_T[f_in, f_out].T @ act[f_in, t]` with the
128x128 weight tile stationary.  Weights are pre-transposed/packed and
cast to bf16 on the host; accumulation is fp32 in PSUM.

Softmax is computed max-free (scores are O(5) for this distribution, so
exp() cannot overflow in fp32): scores are built transposed
([t_k, t_q]) so the attn@V contraction needs no transposes, and the
denominator is the all-ones matmul of the exp tiles.

SBUF tags are hand-shared across phases (a tile pool's footprint is the
static sum over tags), e.g. {x, h_half0, h_half1} rotate through one
32KB/partition slot and {k_sb, v_sb, e_head...} through two 16KB slots.
"""

import math

import ml_dtypes
import numpy as np

import concourse.bass as bass
import concourse.mybir as mybir
import concourse.tile as tile
from concourse import bacc
from concourse.bass import ts
from concourse.bass_utils import run_bass_kernel_spmd

F32 = mybir.dt.float32
BF16 = mybir.dt.bfloat16

B = 2
T = 2048
C = 2048  # hidden
I = 8192  # intermediate
NH = 16
HD = 128
EPS = 1e-6

N_CORES = 8
GROUP = 4  # cores per batch group
T_LOC = (B * T) // N_CORES  # 512 tokens per core
GROUPS = [[0, 1, 2, 3], [4, 5, 6, 7]]

P = 128
NCT = C // P  # 16 c-tiles
NIT = I // P  # 64 i-tiles
NTT = T_LOC // P  # 4 local token tiles
NKT = T // P  # 16 key tiles per batch
IHALF = NIT // 2  # 32 i-tiles per MLP half

_CACHE = {}


def _pack_lhsT(w):
    """[O, C] weight (y = x @ w.T) -> [O//128, 128, C] of stationary tiles.

    block[ot][ci, ct*128 + oi] == w[128*ot + oi, 128*ct + ci], so the
    SBUF tile [128, C] for output-tile `ot` yields lhsT slices
    [:, ct*128 : ct*128+128] = w.T tile with contraction on partitions.
    """
    O, Cin = w.shape
    no, nc_ = O // P, Cin // P
    arr = w.reshape(no, P, nc_, P).transpose(0, 3, 2, 1)  # [ot, ci, ct, oi]
    return np.ascontiguousarray(arr.reshape(no, P, Cin)).astype(ml_dtypes.bfloat16)


def _build_program():
    nc = bacc.Bacc("TRN2", target_bir_lowering=False, debug=False, num_devices=N_CORES)

    x_d = nc.declare_dram_parameter("x", [C, T_LOC], F32, isOutput=False)
    wqT_d = nc.declare_dram_parameter("wqT", [NCT, P, C], BF16, isOutput=False)
    wkT_d = nc.declare_dram_parameter("wkT", [NCT, P, C], BF16, isOutput=False)
    wvT_d = nc.declare_dram_parameter("wvT", [C, C], BF16, isOutput=False)
    woT_d = nc.declare_dram_parameter("woT", [NCT, P, C], BF16, isOutput=False)
    wgT_d = nc.declare_dram_parameter("wgT", [NIT, P, C], BF16, isOutput=False)
    wuT_d = nc.declare_dram_parameter("wuT", [NIT, P, C], BF16, isOutput=False)
    wdT_d = nc.declare_dram_parameter("wdT", [NCT, P, I], BF16, isOutput=False)
    out_d = nc.declare_dram_parameter("out", [C, T_LOC], F32, isOutput=True)

    # collective bounce buffers
    k_cc = nc.dram_tensor("k_cc", [C, T_LOC], BF16)  # local K, feature-major
    v_cc = nc.dram_tensor("v_cc", [T_LOC, C], BF16)  # local V, token-major
    k_ag = nc.dram_tensor("k_ag", [GROUP * C, T_LOC], BF16)
    v_ag = nc.dram_tensor("v_ag", [GROUP * T_LOC, C], BF16)

    with tile.TileContext(nc) as tc:
        with (
            tc.tile_pool(name="sb", bufs=2) as sb,
            tc.tile_pool(name="ps", bufs=2, space="PSUM") as ps,
        ):
            ones_bf = sb.tile([P, P], BF16, tag="ones_bf", bufs=1)
            nc.any.memset(ones_bf, 1.0)
            ones_f32 = sb.tile([P, P], F32, tag="ones_f32", bufs=1)
            nc.any.memset(ones_f32, 1.0)
            eps_t = sb.tile([P, 1], F32, tag="eps", bufs=1)
            nc.any.memset(eps_t, EPS)

            # ---- load x, rmsnorm1 -> x1n (bf16) ----
            # tag "t32": {x, h_half0, h_half1} share one 32KB/part slot
            x_sb = sb.tile([P, NCT, T_LOC], F32, tag="t32", bufs=1, name="x_sb")
            nc.sync.dma_start(
                out=x_sb[:], in_=x_d.rearrange("(ct p) t -> p ct t", p=P)
            )

            def rmsnorm(src_sb, dst_name):
                ssq = ps.tile([P, T_LOC], F32, tag="dn", name="ssq")
                for ct in range(NCT):
                    sq = sb.tile([P, T_LOC], F32, tag="tmp", bufs=3, name="sq")
                    nc.vector.tensor_mul(sq[:], src_sb[:, ct, :], src_sb[:, ct, :])
                    nc.tensor.matmul(
                        ssq[:], ones_f32[:], sq[:], start=(ct == 0), stop=(ct == NCT - 1)
                    )
                rms = sb.tile([P, T_LOC], F32, tag="tmp", bufs=3, name="rms")
                nc.scalar.activation(
                    rms[:], ssq[:], mybir.ActivationFunctionType.Sqrt,
                    bias=eps_t[:], scale=1.0 / C,
                )
                rinv = sb.tile([P, T_LOC], F32, tag="tmp", bufs=3, name="rinv")
                nc.vector.reciprocal(rinv[:], rms[:])
                xn = sb.tile([P, NCT, T_LOC], BF16, tag="xn", bufs=1, name=dst_name)
                for ct in range(NCT):
                    nc.vector.tensor_mul(xn[:, ct, :], src_sb[:, ct, :], rinv[:])
                return xn

            x1n = rmsnorm(x_sb, "x1n")

            # ---- K projection (feature-major) ----
            # tag "se": {k_sb, v_sb, e(head 0), e(head 1), ...} 2 x 16KB slots
            k_sb = sb.tile([P, NCT, T_LOC], BF16, tag="se", bufs=2, name="k_sb")
            for ot in range(NCT):
                wk_t = sb.tile([P, C], BF16, tag="wqk", bufs=2, name="wk_t")
                nc.sync.dma_start(out=wk_t[:], in_=wkT_d[ot])
                k_ps = ps.tile([P, T_LOC], F32, tag="mm", bufs=3, name="k_ps")
                for ct in range(NCT):
                    nc.tensor.matmul(
                        k_ps[:], wk_t[:, ts(ct, P)], x1n[:, ct, :],
                        start=(ct == 0), stop=(ct == NCT - 1),
                    )
                nc.vector.tensor_copy(k_sb[:, ot, :], k_ps[:])
            nc.gpsimd.dma_start(
                out=k_cc.rearrange("(ct p) t -> p ct t", p=P), in_=k_sb[:]
            )

            # ---- V projection (token-major: x1n tiles stationary) ----
            v_sb = sb.tile([P, NTT, C], BF16, tag="se", bufs=2, name="v_sb")
            for nk in range(C // 512):
                v_ps = [
                    ps.tile([P, 512], F32, tag="acc", bufs=2, name="v_ps0"),
                    ps.tile([P, 512], F32, tag="acc", bufs=2, name="v_ps1"),
                    ps.tile([P, 512], F32, tag="dn", bufs=2, name="v_ps2"),
                    ps.tile([P, 512], F32, tag="dn", bufs=2, name="v_ps3"),
                ]
                for ct in range(NCT):
                    wv_t = sb.tile([P, 512], BF16, tag="wvs", bufs=4, name="wv_t")
                    nc.sync.dma_start(
                        out=wv_t[:], in_=wvT_d[ts(ct, P), ts(nk, 512)]
                    )
                    for tt in range(NTT):
                        nc.tensor.matmul(
                            v_ps[tt][:],
                            x1n[:, ct, ts(tt, P)],
                            wv_t[:],
                            start=(ct == 0), stop=(ct == NCT - 1),
                        )
                for tt in range(NTT):
                    nc.vector.tensor_copy(v_sb[:, tt, ts(nk, 512)], v_ps[tt][:])
            nc.gpsimd.dma_start(
                out=v_cc.rearrange("(tt p) o -> p tt o", p=P), in_=v_sb[:]
            )

            # ---- exchange K/V shards within the 4-core batch group ----
            nc.gpsimd.collective_compute(
                "AllGather", mybir.AluOpType.bypass, replica_groups=GROUPS,
                ins=[k_cc[:]], outs=[k_ag[:]],
            )
            nc.gpsimd.collective_compute(
                "AllGather", mybir.AluOpType.bypass, replica_groups=GROUPS,
                ins=[v_cc[:]], outs=[v_ag[:]],
            )

            # ---- Q projection (overlaps the collectives) ----
            q_sb = sb.tile([P, NCT, T_LOC], BF16, tag="q", bufs=1, name="q_sb")
            for ot in range(NCT):
                wq_t = sb.tile([P, C], BF16, tag="wqk", bufs=2, name="wq_t")
                nc.sync.dma_start(out=wq_t[:], in_=wqT_d[ot])
                q_ps = ps.tile([P, T_LOC], F32, tag="mm", bufs=3, name="q_ps")
                for ct in range(NCT):
                    nc.tensor.matmul(
                        q_ps[:], wq_t[:, ts(ct, P)], x1n[:, ct, :],
                        start=(ct == 0), stop=(ct == NCT - 1),
                    )
                nc.vector.tensor_copy(q_sb[:, ot, :], q_ps[:])

            # ---- attention, one head (= one 128-feature tile) at a time ----
            k_ag_v = k_ag.rearrange("(g p) t -> p g t", p=C)  # [2048, 4, 512]
            v_ag_v = v_ag.rearrange("(tc p) d -> p tc d", p=P)  # [128, 16, 2048]
            attn_sb = sb.tile([P, NCT, T_LOC], BF16, tag="attn", bufs=1, name="attn_sb")
            for h in range(NH):
                k_h = sb.tile([P, GROUP, T_LOC], BF16, tag="kh", bufs=2, name="k_h")
                nc.gpsimd.dma_start(out=k_h[:], in_=k_ag_v[ts(h, P), :, :])
                v_h = sb.tile([P, NKT, HD], BF16, tag="vh", bufs=2, name="v_h")
                nc.gpsimd.dma_start(out=v_h[:], in_=v_ag_v[:, :, ts(h, P)])

                e_sb = sb.tile([P, NKT, T_LOC], BF16, tag="se", bufs=2, name="e_sb")
                for c in range(NKT):
                    s_ps = ps.tile([P, T_LOC], F32, tag="mm", bufs=3, name="s_ps")
                    nc.tensor.matmul(
                        s_ps[:],
                        k_h[:, c // NTT, ts(c % NTT, P)],
                        q_sb[:, h, :],
                        start=True, stop=True,
                    )
                    nc.scalar.activation(
                        e_sb[:, c, :], s_ps[:], mybir.ActivationFunctionType.Exp
                    )
                dn_ps = ps.tile([P, T_LOC], F32, tag="dn", bufs=2, name="dn_ps")
                for c in range(NKT):
                    nc.tensor.matmul(
                        dn_ps[:], ones_bf[:], e_sb[:, c, :],
                        start=(c == 0), stop=(c == NKT - 1),
                    )
                av_ps = ps.tile([P, T_LOC], F32, tag="acc", bufs=2, name="av_ps")
                for c in range(NKT):
                    nc.tensor.matmul(
                        av_ps[:], v_h[:, c, :], e_sb[:, c, :],
                        start=(c == 0), stop=(c == NKT - 1),
                    )
                rcp = sb.tile([P, T_LOC], F32, tag="tmp", bufs=3, name="rcp")
                nc.vector.reciprocal(rcp[:], dn_ps[:])
                nc.vector.tensor_mul(attn_sb[:, h, :], av_ps[:], rcp[:])

            # ---- o-proj + residual -> x2 (f32) ----
            x2_sb = sb.tile([P, NCT, T_LOC], F32, tag="x2", bufs=1, name="x2_sb")
            for ot in range(NCT):
                wo_t = sb.tile([P, C], BF16, tag="wqk", bufs=2, name="wo_t")
                nc.sync.dma_start(out=wo_t[:], in_=woT_d[ot])
                o_ps = ps.tile([P, T_LOC], F32, tag="mm", bufs=3, name="o_ps")
                for ct in range(NCT):
                    nc.tensor.matmul(
                        o_ps[:], wo_t[:, ts(ct, P)], attn_sb[:, ct, :],
                        start=(ct == 0), stop=(ct == NCT - 1),
                    )
                nc.vector.tensor_add(x2_sb[:, ot, :], o_ps[:], x_sb[:, ot, :])

            # ---- rmsnorm2 -> x2n ----
            x2n = rmsnorm(x2_sb, "x2n")

            # ---- MLP in two halves of the intermediate dim ----
            for half in range(2):
                h_sb = sb.tile(
                    [P, IHALF, T_LOC], BF16, tag="t32", bufs=1, name=f"h_sb{half}"
                )
                for ii in range(IHALF):
                    it = half * IHALF + ii
                    wg_t = sb.tile([P, C], BF16, tag="wgu", bufs=2, name="wg_t")
                    nc.sync.dma_start(out=wg_t[:], in_=wgT_d[it])
                    wu_t = sb.tile([P, C], BF16, tag="wgu", bufs=2, name="wu_t")
                    nc.sync.dma_start(out=wu_t[:], in_=wuT_d[it])
                    g_ps = ps.tile([P, T_LOC], F32, tag="mm", bufs=3, name="g_ps")
                    for ct in range(NCT):
                        nc.tensor.matmul(
                            g_ps[:], wg_t[:, ts(ct, P)], x2n[:, ct, :],
                            start=(ct == 0), stop=(ct == NCT - 1),
                        )
                    u_ps = ps.tile([P, T_LOC], F32, tag="mm", bufs=3, name="u_ps")
                    for ct in range(NCT):
                        nc.tensor.matmul(
                            u_ps[:], wu_t[:, ts(ct, P)], x2n[:, ct, :],
                            start=(ct == 0), stop=(ct == NCT - 1),
                        )
                    g_sb = sb.tile([P, T_LOC], BF16, tag="gs", bufs=2, name="g_sb")
                    nc.scalar.activation(
                        g_sb[:], g_ps[:], mybir.ActivationFunctionType.Silu
                    )
                    nc.vector.tensor_mul(h_sb[:, ii, :], u_ps[:], g_sb[:])

                # down-proj of this half, accumulated into x2 in place
                for ot in range(NCT):
                    wd_t = sb.tile([P, IHALF * P], BF16, tag="wd", bufs=2, name="wd_t")
                    nc.sync.dma_start(
                        out=wd_t[:], in_=wdT_d[ot][:, ts(half, IHALF * P)]
                    )
                    y_ps = ps.tile([P, T_LOC], F32, tag="acc", bufs=2, name="y_ps")
                    for ii in range(IHALF):
                        nc.tensor.matmul(
                            y_ps[:], wd_t[:, ts(ii, P)], h_sb[:, ii, :],
                            start=(ii == 0), stop=(ii == IHALF - 1),
                        )
                    nc.vector.tensor_add(x2_sb[:, ot, :], y_ps[:], x2_sb[:, ot, :])

            for ot in range(NCT):
                nc.sync.dma_start(out=out_d[ts(ot, P), :], in_=x2_sb[:, ot, :])

    nc.compile()
    return nc


def _pack_inputs(x, w_ln1, wq, wk, wv, wo, w_ln2, wg, wu, wd):
    scale = 1.0 / math.sqrt(HD)
    wq_eff = (wq * w_ln1[None, :]) * scale
    wk_eff = wk * w_ln1[None, :]
    wv_eff = wv * w_ln1[None, :]
    wg_eff = wg * w_ln2[None, :]
    wu_eff = wu * w_ln2[None, :]

    weights = {
        "wqT": _pack_lhsT(wq_eff),
        "wkT": _pack_lhsT(wk_eff),
        "wvT": np.ascontiguousarray(wv_eff.T).astype(ml_dtypes.bfloat16),
        "woT": _pack_lhsT(wo),
        "wgT": _pack_lhsT(wg_eff),
        "wuT": _pack_lhsT(wu_eff),
        "wdT": _pack_lhsT(wd),
    }
    in_maps = []
    for core in range(N_CORES):
        b = core // GROUP
        t0 = (core % GROUP) * T_LOC
        x_loc = np.ascontiguousarray(x[b, t0 : t0 + T_LOC, :].T).astype(np.float32)
        in_maps.append({"x": x_loc, **weights})
    return in_maps


def kernel(**inputs):
    if "nc" not in _CACHE:
        _CACHE["nc"] = _build_program()
    nc = _CACHE["nc"]
    in_maps = _pack_inputs(**inputs)
    res = run_bass_kernel_spmd(nc, in_maps, core_ids=list(range(N_CORES)))
    out = np.empty((B, T, C), dtype=np.float32)
    for core in range(N_CORES):
        b = core // GROUP
        t0 = (core % GROUP) * T_LOC
        out[b, t0 : t0 + T_LOC, :] = res.results[core]["out"].T
    return out
